# revision 27
# baseline (speedup 1.0000x reference)
"""Multi-head self-attention (B=16,T=512,C=1024,H=16) on 8 NeuronCores.

Strategy: data-parallel over batch (2 batches/core), no collectives.
All matmuls run in float32r (full PE rate at moving-dim >= 256).
Layout is chosen so no on-device transposes are needed:
  - QK projection emits [f, tok] (q^T / k^T per head are direct slices)
  - V projection swaps matmul operands to emit v as [tok, f]
  - scores are computed transposed: sT[kt, qt]; softmax sums arrive via a
    ones-column appended to v in the AV matmul; masking is a 0/1 multiply
    after exp (exact, since exp>0 and rows are never fully masked).
  - normalization (1/l) is broadcast across partitions via a DRAM bounce
    and folded into the PSUM->SBUF copy of the attention output.
"""

import math

import numpy as np

import concourse.bass as bass
import concourse.mybir as mybir
import concourse.tile as tile
from concourse import bacc
from concourse.bass_utils import run_bass_kernel_spmd

N_CORES = 8
B, T, C = 16, 512, 1024
H = 16
DH = C // H  # 64
B_LOC = B // N_CORES  # 2
TOK = B_LOC * T  # 1024 tokens per core
P = 128
CT = C // P  # 8 contraction tiles
FQK = 2 * C  # q+k rows
DT = mybir.dt.float16
F32 = mybir.dt.float32


def _build_nc():
    nc = bacc.Bacc("TRN2", target_bir_lowering=False, debug=False,
                   num_devices=N_CORES)

    xT = nc.dram_tensor("xT", [C, TOK], DT, kind="ExternalInput").ap()
    wqkT = nc.dram_tensor("wqkT", [C, FQK], DT, kind="ExternalInput").ap()
    wvT = nc.dram_tensor("wvT", [C, C], DT, kind="ExternalInput").ap()
    woT = nc.dram_tensor("woT", [C, C], DT, kind="ExternalInput").ap()
    maskd = nc.dram_tensor("maskd", [T // P, P, P], DT,
                           kind="ExternalInput").ap()
    kpmb = nc.dram_tensor("kpmb", [B_LOC, T], F32, kind="ExternalInput").ap()
    bias = nc.dram_tensor("bias", [C], F32, kind="ExternalInput").ap()
    out = nc.dram_tensor("out", [TOK, C], F32, kind="ExternalOutput").ap()
    lall = nc.dram_tensor("lall", [B_LOC, H, T], F32).ap()
    linv_scr = nc.dram_tensor("linv_scr", [B_LOC, H, T], DT).ap()

    with tile.TileContext(nc) as tc:
        _emit(nc, tc, xT, wqkT, wvT, woT, maskd, kpmb, bias, out, lall,
              linv_scr)

    nc.compile()
    return nc


def _emit(nc, tc, xT, wqkT, wvT, woT, maskd, kpmb, bias, out, lall, linv_scr):
    from contextlib import ExitStack
    ctx = ExitStack()
    with ctx:
        singles = ctx.enter_context(tc.tile_pool(name="singles", bufs=1))
        wo_pool = ctx.enter_context(tc.tile_pool(name="wo", bufs=1))
        ps_a = ctx.enter_context(tc.tile_pool(name="ps_a", bufs=4, space="PSUM"))
        ps_s = ctx.enter_context(tc.tile_pool(name="ps_s", bufs=2, space="PSUM"))
        ps_o = ctx.enter_context(tc.tile_pool(name="ps_o", bufs=2, space="PSUM"))
        pt_pool = ctx.enter_context(tc.tile_pool(name="pt", bufs=2))
        linv_pool = ctx.enter_context(tc.tile_pool(name="linv", bufs=2))
        ao_stage_pool = ctx.enter_context(tc.tile_pool(name="aost", bufs=2))
        y_pool = ctx.enter_context(tc.tile_pool(name="y", bufs=3))

        NR = T // P  # 4 kt blocks

        # --- persistent SBUF tensors ---
        qk_sb = singles.tile([P, 16, TOK], DT)             # 32 KB/part
        v_sb = singles.tile([P, TOK // P, H, DH + 1], DT)  # 16.6 KB/part
        ao_k = [[singles.tile([P, T], DT, name=f"ao_{b}_{k}")
                 for k in range(CT)] for b in range(B_LOC)]  # 16 KB/part
        lpa = [[singles.tile([H // 2, T], F32, name=f"lpa_{b}_{hf}")
                for hf in range(2)] for b in range(B_LOC)]
        bias_sb = singles.tile([P, C], F32)                # 4 KB/part
        maskd_sb = singles.tile([P, NR, P], DT)            # 1 KB/part
        kpmb_sb = singles.tile([P, B_LOC * NR], F32)

        with tc.tile_pool(name="xp", bufs=1) as x_pool, \
                tc.tile_pool(name="wq", bufs=3) as wq_pool, \
                tc.tile_pool(name="wv", bufs=1) as wv_pool:
            xk = [x_pool.tile([P, TOK], DT, tag=f"x_{k}", name=f"x_{k}")
                  for k in range(CT)]
            # first x chunk + first weight tile up front so the PE can start
            nc.sync.dma_start(out=xk[0][:], in_=xT[0:P, :])
            wq0 = wq_pool.tile([P, CT, P], DT, tag="wq", name="wq_0")
            nc.sync.dma_start(
                out=wq0[:],
                in_=wqkT[:, 0:P].rearrange("(k p) f -> p k f", p=P))
            for k in range(1, CT):
                nc.sync.dma_start(out=xk[k][:],
                                  in_=xT[k * P:(k + 1) * P, :])

            # --- phase 1: QK projection -> qk_sb[f, tok] ---
            for j in range(16):
                if j == 0:
                    wq = wq0
                else:
                    wq = wq_pool.tile([P, CT, P], DT, tag="wq",
                                      name=f"wq_{j}")
                    nc.sync.dma_start(
                        out=wq[:],
                        in_=wqkT[:, j * P:(j + 1) * P].rearrange(
                            "(k p) f -> p k f", p=P))
                ps = [ps_a.tile([P, 512], F32, tag="ps_a", name=f"ps_qk_{j}_{tt}")
                      for tt in range(2)]
                for k in range(CT):
                    for tt in range(2):
                        nc.tensor.matmul(ps[tt][:], wq[:, k, :],
                                         xk[k][:, tt * 512:(tt + 1) * 512],
                                         start=(k == 0), stop=(k == CT - 1))
                for tt in range(2):
                    nc.vector.tensor_copy(
                        out=qk_sb[:, j, tt * 512:(tt + 1) * 512],
                        in_=ps[tt][:])

            # --- phase 2: V projection -> v_sb[tok, h, d] (+ ones col) ---
            nc.vector.memset(v_sb[:, :, :, DH:DH + 1], 1.0)
            for n in range(2):
                wv = [wv_pool.tile([P, 512], DT, tag=f"wv_{k}",
                                   name=f"wv_{n}_{k}") for k in range(CT)]
                for k in range(CT):
                    nc.sync.dma_start(
                        out=wv[k][:],
                        in_=wvT[k * P:(k + 1) * P, n * 512:(n + 1) * 512])
                for m in range(TOK // P):
                    ps = ps_a.tile([P, 512], F32, tag="ps_a", name=f"ps_v_{n}_{m}")
                    for k in range(CT):
                        nc.tensor.matmul(
                            ps[:], xk[k][:, m * P:(m + 1) * P], wv[k][:],
                            start=(k == 0), stop=(k == CT - 1))
                    nc.vector.tensor_copy(
                        out=v_sb[:, m, 8 * n:8 * n + 8, 0:DH],
                        in_=ps[:].rearrange("p (h d) -> p h d", d=DH))

        # small constants + out-projection weights (prefetched here so they
        # don't delay the projection phase DMAs)
        nc.sync.dma_start(out=maskd_sb[:],
                          in_=maskd.rearrange("r p q -> p r q"))
        nc.sync.dma_start(out=kpmb_sb[:],
                          in_=kpmb.rearrange("b (r p) -> p (b r)", p=P))
        bias_bcast = bass.AP(tensor=bias.tensor, offset=bias.offset,
                             ap=[[0, P], *bias.ap])
        nc.gpsimd.dma_start(out=bias_sb[:], in_=bias_bcast)
        wo = [wo_pool.tile([P, 512], DT, tag=f"wo_{n}_{k}", name=f"wo_{n}_{k}")
              for n in range(2) for k in range(CT)]
        for n in range(2):
            for k in range(CT):
                nc.sync.dma_start(
                    out=wo[n * CT + k][:],
                    in_=woT[k * P:(k + 1) * P, n * 512:(n + 1) * 512])

        def yproj_chunk(b, i):
            n, m = i // (T // P), i % (T // P)
            ps = ps_a.tile([P, 512], F32, tag="ps_a", name=f"ps_y_{b}_{n}_{m}")
            for k in range(CT):
                nc.tensor.matmul(ps[:], ao_k[b][k][:, m * P:(m + 1) * P],
                                 wo[n * CT + k][:],
                                 start=(k == 0), stop=(k == CT - 1))
            y = y_pool.tile([P, 512], F32, tag="y")
            nc.vector.tensor_add(out=y[:], in0=ps[:],
                                 in1=bias_sb[:, n * 512:(n + 1) * 512])
            nc.sync.dma_start(
                out=out[b * T + m * P: b * T + (m + 1) * P,
                        n * 512:(n + 1) * 512],
                in_=y[:])

        # --- phase 3+4 interleaved per batch: b0 attention, then b1
        # attention with b0's out-projection chunks woven between heads ---
        for b in range(B_LOC):
            for h in range(H):
                jq, jk, dlo = h // 2, 8 + h // 2, DH * (h % 2)
                pT = pt_pool.tile([P, NR, 512], DT, tag="pT")
                sT = [ps_s.tile([P, 512], F32, tag="sT", name=f"sT_{b}_{h}_{r}")
                      for r in range(NR)]
                for r in range(NR):
                    kT = qk_sb[dlo:dlo + DH, jk,
                               b * T + r * P: b * T + (r + 1) * P]
                    qTr = qk_sb[dlo:dlo + DH, jq,
                                b * T + r * P:(b + 1) * T]
                    nc.tensor.matmul(sT[r][:, r * P:], kT, qTr,
                                     start=True, stop=True)
                    # exp over the un-masked tail; key-padding enters as an
                    # additive bias (0 or -1e30) per kt partition
                    nc.scalar.activation(
                        out=pT[:, r, r * P:], in_=sT[r][:, r * P:],
                        func=mybir.ActivationFunctionType.Exp,
                        bias=kpmb_sb[:, b * NR + r: b * NR + r + 1])
                    # causal mask inside the diagonal block only
                    nc.vector.tensor_mul(
                        out=pT[:, r, r * P:(r + 1) * P],
                        in0=pT[:, r, r * P:(r + 1) * P],
                        in1=maskd_sb[:, r, :])
                po = ps_o.tile([P, 512], F32, tag="po")
                for r in range(NR):
                    # masked columns of pT are never read: slice rhs/out
                    nc.tensor.matmul(po[0:DH + 1, r * P:],
                                     v_sb[:, b * NR + r, h, :],
                                     pT[:, r, r * P:],
                                     start=(r == 0), stop=(r == NR - 1))
                # stash row sums; normalization is batched per b
                lrow = linv_pool.tile([P, 512], F32, tag="lrow")
                nc.vector.tensor_copy(out=lrow[DH:DH + 1, :],
                                      in_=po[DH:DH + 1, :])
                nc.sync.dma_start(
                    out=lpa[b][h // (H // 2)][h % (H // 2): h % (H // 2) + 1, :],
                    in_=lrow[DH:DH + 1, :])
                if dlo == 0:
                    nc.vector.tensor_copy(
                        out=ao_k[b][jq][0:DH, :], in_=po[0:DH, :])
                else:
                    ao_st = ao_stage_pool.tile([DH, 512], DT, tag="ao_st")
                    nc.vector.tensor_copy(out=ao_st[:], in_=po[0:DH, :])
                    nc.sync.dma_start(out=ao_k[b][jq][dlo:dlo + DH, :],
                                      in_=ao_st[:])
                if b == 1 and h % 2 == 1:
                    yproj_chunk(0, h // 2)
                if h % (H // 2) == H // 2 - 1:
                    # 1/l for the finished half of the heads, then in-place
                    # normalize the corresponding ao c-tiles
                    half = h // (H // 2)
                    hs = slice(half * (H // 2), (half + 1) * (H // 2))
                    nc.vector.reciprocal(out=lpa[b][half][:],
                                         in_=lpa[b][half][:])
                    lpartd = linv_pool.tile([H // 2, T], DT, tag="lpartd",
                                            name=f"lpartd_{b}_{half}")
                    nc.vector.tensor_copy(out=lpartd[:], in_=lpa[b][half][:])
                    nc.sync.dma_start(out=linv_scr[b, hs], in_=lpartd[:])
                    for k in range(half * (CT // 2), (half + 1) * (CT // 2)):
                        lf = linv_pool.tile([P, T], DT, tag="lf")
                        for hf in range(2):
                            hh = 2 * k + hf
                            src_ap = bass.AP(
                                tensor=linv_scr.tensor,
                                offset=linv_scr.offset + (b * H + hh) * T,
                                ap=[[0, DH], [1, T]])
                            nc.sync.dma_start(
                                out=lf[hf * DH:(hf + 1) * DH, :], in_=src_ap)
                        nc.vector.tensor_mul(out=ao_k[b][k][:],
                                             in0=ao_k[b][k][:], in1=lf[:])

        for i in range(2 * (T // P)):
            yproj_chunk(1, i)





_NC_CACHE = None


def _get_nc():
    global _NC_CACHE
    if _NC_CACHE is None:
        _NC_CACHE = _build_nc()
    return _NC_CACHE


def _prep_core_inputs(x, mask, key_padding_mask, w_qkv, w_out, b_out):
    """Host-side sharding + layout prep. Returns list of per-core in_maps."""
    x = np.asarray(x, dtype=np.float32)
    mask = np.asarray(mask)
    kpm = np.asarray(key_padding_mask)
    w_qkv = np.asarray(w_qkv, dtype=np.float32)
    w_out = np.asarray(w_out, dtype=np.float32)
    b_out = np.asarray(b_out, dtype=np.float32)

    scale = 1.0 / math.sqrt(DH)
    wqkT = w_qkv[:FQK].T.copy()  # [C, 2C]
    wqkT[:, :C] *= scale  # fold 1/sqrt(dh) into the Q weights
    wqkT = wqkT.astype(np.float16)
    wvT = np.ascontiguousarray(w_qkv[FQK:].T.astype(np.float16))  # [C, C]
    woT = np.ascontiguousarray(w_out.T.astype(np.float16))        # [C, C]

    # The kernel exploits the causal structure: it only applies mask values
    # inside the diagonal 128x128 blocks and zero-fills fully-masked blocks.
    # Verify the input mask really is lower-triangular.
    NRl = T // P
    exp_tril = np.tril(np.ones((T, T), dtype=mask.dtype))
    assert np.array_equal(mask, exp_tril), "kernel assumes causal tril mask"
    maskTf = mask.T.astype(np.float16)  # [kt, qt]
    maskd = np.stack([maskTf[r * P:(r + 1) * P, r * P:(r + 1) * P]
                      for r in range(NRl)])  # [NR, P, P]

    in_maps = []
    for i in range(N_CORES):
        xs = x[i * B_LOC:(i + 1) * B_LOC]      # [B_LOC, T, C]
        xT = np.ascontiguousarray(xs.reshape(TOK, C).T.astype(np.float16))
        kb = np.where(kpm[i * B_LOC:(i + 1) * B_LOC], -1e30,
                      0.0).astype(np.float32)  # [B_LOC, T]
        in_maps.append({
            "xT": xT,
            "wqkT": wqkT,
            "wvT": wvT,
            "woT": woT,
            "maskd": np.ascontiguousarray(maskd),
            "kpmb": kb,
            "bias": b_out,
        })
    return in_maps


def kernel(x, mask, key_padding_mask, w_qkv, w_out, b_out, _trace=False,
           _tmpdir=None):
    nc = _get_nc()
    in_maps = _prep_core_inputs(x, mask, key_padding_mask, w_qkv, w_out, b_out)
    res = run_bass_kernel_spmd(nc, in_maps, list(range(N_CORES)),
                               trace=_trace, tmpdir=_tmpdir)
    outs = [res.results[i]["out"].reshape(B_LOC, T, C) for i in range(N_CORES)]
    full = np.concatenate(outs, axis=0).astype(np.float32)
    kernel._last_exec_time_ns = res.exec_time_ns
    return full


# revision 28
# speedup vs baseline: 1.1504x; 1.1504x over previous
"""Multi-head self-attention (B=16,T=512,C=1024,H=16) on 8 NeuronCores.

Strategy: data-parallel over batch (2 batches/core), no collectives.
All matmuls run in float32r (full PE rate at moving-dim >= 256).
Layout is chosen so no on-device transposes are needed:
  - QK projection emits [f, tok] (q^T / k^T per head are direct slices)
  - V projection swaps matmul operands to emit v as [tok, f]
  - scores are computed transposed: sT[kt, qt]; softmax sums arrive via a
    ones-column appended to v in the AV matmul; masking is a 0/1 multiply
    after exp (exact, since exp>0 and rows are never fully masked).
  - normalization (1/l) is broadcast across partitions via a DRAM bounce
    and folded into the PSUM->SBUF copy of the attention output.
"""

import math

import numpy as np

import concourse.bass as bass
import concourse.mybir as mybir
import concourse.tile as tile
from concourse import bacc
from concourse.bass_utils import run_bass_kernel_spmd

N_CORES = 8
B, T, C = 16, 512, 1024
H = 16
DH = C // H  # 64
B_LOC = B // N_CORES  # 2
TOK = B_LOC * T  # 1024 tokens per core
P = 128
CT = C // P  # 8 contraction tiles
FQK = 2 * C  # q+k rows
DT = mybir.dt.float16
F32 = mybir.dt.float32


def _build_nc():
    nc = bacc.Bacc("TRN2", target_bir_lowering=False, debug=False,
                   num_devices=N_CORES)

    xT = nc.dram_tensor("xT", [C, TOK], DT, kind="ExternalInput").ap()
    wqkT = nc.dram_tensor("wqkT", [C, FQK], DT, kind="ExternalInput").ap()
    wvT = nc.dram_tensor("wvT", [C, C], DT, kind="ExternalInput").ap()
    woT = nc.dram_tensor("woT", [C, C], DT, kind="ExternalInput").ap()
    maskd = nc.dram_tensor("maskd", [T // P, P, P], DT,
                           kind="ExternalInput").ap()
    kpmb = nc.dram_tensor("kpmb", [B_LOC, T], F32, kind="ExternalInput").ap()
    bias = nc.dram_tensor("bias", [C], F32, kind="ExternalInput").ap()
    out = nc.dram_tensor("out", [TOK, C], F32, kind="ExternalOutput").ap()
    lall = nc.dram_tensor("lall", [B_LOC, H, T], F32).ap()
    linv_scr = nc.dram_tensor("linv_scr", [B_LOC, H, T], DT).ap()

    with tile.TileContext(nc) as tc:
        _emit(nc, tc, xT, wqkT, wvT, woT, maskd, kpmb, bias, out, lall,
              linv_scr)

    nc.compile()
    return nc


def _emit(nc, tc, xT, wqkT, wvT, woT, maskd, kpmb, bias, out, lall, linv_scr):
    from contextlib import ExitStack
    ctx = ExitStack()
    with ctx:
        singles = ctx.enter_context(tc.tile_pool(name="singles", bufs=1))
        wo_pool = ctx.enter_context(tc.tile_pool(name="wo", bufs=1))
        ps_a = ctx.enter_context(tc.tile_pool(name="ps_a", bufs=4, space="PSUM"))
        ps_s = ctx.enter_context(tc.tile_pool(name="ps_s", bufs=2, space="PSUM"))
        ps_o = ctx.enter_context(tc.tile_pool(name="ps_o", bufs=2, space="PSUM"))
        pt_pool = ctx.enter_context(tc.tile_pool(name="pt", bufs=2))
        linv_pool = ctx.enter_context(tc.tile_pool(name="linv", bufs=2))
        ao_stage_pool = ctx.enter_context(tc.tile_pool(name="aost", bufs=2))
        y_pool = ctx.enter_context(tc.tile_pool(name="y", bufs=3))

        NR = T // P  # 4 kt blocks

        # --- persistent SBUF tensors ---
        qk_sb = singles.tile([P, 16, TOK], DT)             # 32 KB/part
        v_sb = singles.tile([P, TOK // P, H, DH + 1], DT)  # 16.6 KB/part
        ao_k = [[singles.tile([P, T], DT, name=f"ao_{b}_{k}")
                 for k in range(CT)] for b in range(B_LOC)]  # 16 KB/part

        bias_sb = singles.tile([P, C], F32)                # 4 KB/part
        maskd_sb = singles.tile([P, NR, P], DT)            # 1 KB/part
        kpmb_sb = singles.tile([P, B_LOC * NR], F32)

        with tc.tile_pool(name="xp", bufs=1) as x_pool, \
                tc.tile_pool(name="wq", bufs=3) as wq_pool, \
                tc.tile_pool(name="wv", bufs=1) as wv_pool:
            xk = [x_pool.tile([P, TOK], DT, tag=f"x_{k}", name=f"x_{k}")
                  for k in range(CT)]
            # first x chunk + first weight tile up front so the PE can start
            nc.sync.dma_start(out=xk[0][:], in_=xT[0:P, :])
            wq0 = wq_pool.tile([P, CT, P], DT, tag="wq", name="wq_0")
            nc.sync.dma_start(
                out=wq0[:],
                in_=wqkT[:, 0:P].rearrange("(k p) f -> p k f", p=P))
            for k in range(1, CT):
                nc.sync.dma_start(out=xk[k][:],
                                  in_=xT[k * P:(k + 1) * P, :])

            # --- phase 1: QK projection -> qk_sb[f, tok] ---
            for j in range(16):
                if j == 0:
                    wq = wq0
                else:
                    wq = wq_pool.tile([P, CT, P], DT, tag="wq",
                                      name=f"wq_{j}")
                    nc.sync.dma_start(
                        out=wq[:],
                        in_=wqkT[:, j * P:(j + 1) * P].rearrange(
                            "(k p) f -> p k f", p=P))
                ps = [ps_a.tile([P, 512], F32, tag="ps_a", name=f"ps_qk_{j}_{tt}")
                      for tt in range(2)]
                for k in range(CT):
                    for tt in range(2):
                        nc.tensor.matmul(ps[tt][:], wq[:, k, :],
                                         xk[k][:, tt * 512:(tt + 1) * 512],
                                         start=(k == 0), stop=(k == CT - 1))
                for tt in range(2):
                    nc.vector.tensor_copy(
                        out=qk_sb[:, j, tt * 512:(tt + 1) * 512],
                        in_=ps[tt][:])

            # --- phase 2: V projection -> v_sb[tok, h, d] (+ ones col) ---
            nc.vector.memset(v_sb[:, :, :, DH:DH + 1], 1.0)
            for n in range(2):
                wv = [wv_pool.tile([P, 512], DT, tag=f"wv_{k}",
                                   name=f"wv_{n}_{k}") for k in range(CT)]
                for k in range(CT):
                    nc.sync.dma_start(
                        out=wv[k][:],
                        in_=wvT[k * P:(k + 1) * P, n * 512:(n + 1) * 512])
                for m in range(TOK // P):
                    ps = ps_a.tile([P, 512], F32, tag="ps_a", name=f"ps_v_{n}_{m}")
                    for k in range(CT):
                        nc.tensor.matmul(
                            ps[:], xk[k][:, m * P:(m + 1) * P], wv[k][:],
                            start=(k == 0), stop=(k == CT - 1))
                    nc.vector.tensor_copy(
                        out=v_sb[:, m, 8 * n:8 * n + 8, 0:DH],
                        in_=ps[:].rearrange("p (h d) -> p h d", d=DH))

        # small constants + out-projection weights (prefetched here so they
        # don't delay the projection phase DMAs)
        nc.sync.dma_start(out=maskd_sb[:],
                          in_=maskd.rearrange("r p q -> p r q"))
        nc.sync.dma_start(out=kpmb_sb[:],
                          in_=kpmb.rearrange("b (r p) -> p (b r)", p=P))
        bias_bcast = bass.AP(tensor=bias.tensor, offset=bias.offset,
                             ap=[[0, P], *bias.ap])
        nc.gpsimd.dma_start(out=bias_sb[:], in_=bias_bcast)
        wo = [wo_pool.tile([P, 512], DT, tag=f"wo_{n}_{k}", name=f"wo_{n}_{k}")
              for n in range(2) for k in range(CT)]
        for n in range(2):
            for k in range(CT):
                nc.sync.dma_start(
                    out=wo[n * CT + k][:],
                    in_=woT[k * P:(k + 1) * P, n * 512:(n + 1) * 512])

        def yproj_chunk(b, i):
            n, m = i // (T // P), i % (T // P)
            ps = ps_a.tile([P, 512], F32, tag="ps_a", name=f"ps_y_{b}_{n}_{m}")
            for k in range(CT):
                nc.tensor.matmul(ps[:], ao_k[b][k][:, m * P:(m + 1) * P],
                                 wo[n * CT + k][:],
                                 start=(k == 0), stop=(k == CT - 1))
            y = y_pool.tile([P, 512], F32, tag="y")
            nc.vector.tensor_add(out=y[:], in0=ps[:],
                                 in1=bias_sb[:, n * 512:(n + 1) * 512])
            nc.sync.dma_start(
                out=out[b * T + m * P: b * T + (m + 1) * P,
                        n * 512:(n + 1) * 512],
                in_=y[:])

        # --- phase 3+4 interleaved per batch: b0 attention, then b1
        # attention with b0's out-projection chunks woven between heads ---
        for b in range(B_LOC):
            for h in range(H):
                jq, jk, dlo = h // 2, 8 + h // 2, DH * (h % 2)
                pT = pt_pool.tile([P, NR, 512], DT, tag="pT")
                sT = [ps_s.tile([P, 512], F32, tag="sT", name=f"sT_{b}_{h}_{r}")
                      for r in range(NR)]
                for r in range(NR):
                    kT = qk_sb[dlo:dlo + DH, jk,
                               b * T + r * P: b * T + (r + 1) * P]
                    qTr = qk_sb[dlo:dlo + DH, jq,
                                b * T + r * P:(b + 1) * T]
                    nc.tensor.matmul(sT[r][:, r * P:], kT, qTr,
                                     start=True, stop=True)
                    # exp over the un-masked tail; key-padding enters as an
                    # additive bias (0 or -1e30) per kt partition
                    nc.scalar.activation(
                        out=pT[:, r, r * P:], in_=sT[r][:, r * P:],
                        func=mybir.ActivationFunctionType.Exp,
                        bias=kpmb_sb[:, b * NR + r: b * NR + r + 1])
                    # causal mask inside the diagonal block only
                    nc.vector.tensor_mul(
                        out=pT[:, r, r * P:(r + 1) * P],
                        in0=pT[:, r, r * P:(r + 1) * P],
                        in1=maskd_sb[:, r, :])
                po = ps_o.tile([P, 512], F32, tag="po")
                for r in range(NR):
                    # masked columns of pT are never read: slice rhs/out
                    nc.tensor.matmul(po[0:DH + 1, r * P:],
                                     v_sb[:, b * NR + r, h, :],
                                     pT[:, r, r * P:],
                                     start=(r == 0), stop=(r == NR - 1))
                # stash row sums; normalization is batched per b
                lrow = linv_pool.tile([P, 512], F32, tag="lrow")
                nc.vector.tensor_copy(out=lrow[DH:DH + 1, :],
                                      in_=po[DH:DH + 1, :])
                nc.sync.dma_start(out=lall[b, h, :], in_=lrow[DH:DH + 1, :])
                if dlo == 0:
                    nc.vector.tensor_copy(
                        out=ao_k[b][jq][0:DH, :], in_=po[0:DH, :])
                else:
                    ao_st = ao_stage_pool.tile([DH, 512], DT, tag="ao_st")
                    nc.vector.tensor_copy(out=ao_st[:], in_=po[0:DH, :])
                    nc.sync.dma_start(out=ao_k[b][jq][dlo:dlo + DH, :],
                                      in_=ao_st[:])
                if b == 1 and h % 2 == 1:
                    yproj_chunk(0, h // 2)
                if h % (H // 2) == H // 2 - 1:
                    # 1/l for the finished half of the heads, then in-place
                    # normalize the corresponding ao c-tiles
                    half = h // (H // 2)
                    hs = slice(half * (H // 2), (half + 1) * (H // 2))
                    lpart = linv_pool.tile([H // 2, T], F32, tag="lpart",
                                           name=f"lpart_{b}_{half}")
                    nc.sync.dma_start(out=lpart[:], in_=lall[b, hs])
                    nc.vector.reciprocal(out=lpart[:], in_=lpart[:])
                    lpartd = linv_pool.tile([H // 2, T], DT, tag="lpartd",
                                            name=f"lpartd_{b}_{half}")
                    nc.vector.tensor_copy(out=lpartd[:], in_=lpart[:])
                    nc.sync.dma_start(out=linv_scr[b, hs], in_=lpartd[:])
                    for k in range(half * (CT // 2), (half + 1) * (CT // 2)):
                        lf = linv_pool.tile([P, T], DT, tag="lf")
                        for hf in range(2):
                            hh = 2 * k + hf
                            src_ap = bass.AP(
                                tensor=linv_scr.tensor,
                                offset=linv_scr.offset + (b * H + hh) * T,
                                ap=[[0, DH], [1, T]])
                            nc.sync.dma_start(
                                out=lf[hf * DH:(hf + 1) * DH, :], in_=src_ap)
                        nc.vector.tensor_mul(out=ao_k[b][k][:],
                                             in0=ao_k[b][k][:], in1=lf[:])

        for i in range(2 * (T // P)):
            yproj_chunk(1, i)





_NC_CACHE = None


def _get_nc():
    global _NC_CACHE
    if _NC_CACHE is None:
        _NC_CACHE = _build_nc()
    return _NC_CACHE


def _prep_core_inputs(x, mask, key_padding_mask, w_qkv, w_out, b_out):
    """Host-side sharding + layout prep. Returns list of per-core in_maps."""
    x = np.asarray(x, dtype=np.float32)
    mask = np.asarray(mask)
    kpm = np.asarray(key_padding_mask)
    w_qkv = np.asarray(w_qkv, dtype=np.float32)
    w_out = np.asarray(w_out, dtype=np.float32)
    b_out = np.asarray(b_out, dtype=np.float32)

    scale = 1.0 / math.sqrt(DH)
    wqkT = w_qkv[:FQK].T.copy()  # [C, 2C]
    wqkT[:, :C] *= scale  # fold 1/sqrt(dh) into the Q weights
    wqkT = wqkT.astype(np.float16)
    wvT = np.ascontiguousarray(w_qkv[FQK:].T.astype(np.float16))  # [C, C]
    woT = np.ascontiguousarray(w_out.T.astype(np.float16))        # [C, C]

    # The kernel exploits the causal structure: it only applies mask values
    # inside the diagonal 128x128 blocks and zero-fills fully-masked blocks.
    # Verify the input mask really is lower-triangular.
    NRl = T // P
    exp_tril = np.tril(np.ones((T, T), dtype=mask.dtype))
    assert np.array_equal(mask, exp_tril), "kernel assumes causal tril mask"
    maskTf = mask.T.astype(np.float16)  # [kt, qt]
    maskd = np.stack([maskTf[r * P:(r + 1) * P, r * P:(r + 1) * P]
                      for r in range(NRl)])  # [NR, P, P]

    in_maps = []
    for i in range(N_CORES):
        xs = x[i * B_LOC:(i + 1) * B_LOC]      # [B_LOC, T, C]
        xT = np.ascontiguousarray(xs.reshape(TOK, C).T.astype(np.float16))
        kb = np.where(kpm[i * B_LOC:(i + 1) * B_LOC], -1e30,
                      0.0).astype(np.float32)  # [B_LOC, T]
        in_maps.append({
            "xT": xT,
            "wqkT": wqkT,
            "wvT": wvT,
            "woT": woT,
            "maskd": np.ascontiguousarray(maskd),
            "kpmb": kb,
            "bias": b_out,
        })
    return in_maps


def kernel(x, mask, key_padding_mask, w_qkv, w_out, b_out, _trace=False,
           _tmpdir=None):
    nc = _get_nc()
    in_maps = _prep_core_inputs(x, mask, key_padding_mask, w_qkv, w_out, b_out)
    res = run_bass_kernel_spmd(nc, in_maps, list(range(N_CORES)),
                               trace=_trace, tmpdir=_tmpdir)
    outs = [res.results[i]["out"].reshape(B_LOC, T, C) for i in range(N_CORES)]
    full = np.concatenate(outs, axis=0).astype(np.float32)
    kernel._last_exec_time_ns = res.exec_time_ns
    return full


# revision 29
# speedup vs baseline: 1.1926x; 1.0366x over previous
"""Multi-head self-attention (B=16,T=512,C=1024,H=16) on 8 NeuronCores.

Strategy: data-parallel over batch (2 batches/core), no collectives.
All matmuls run in float32r (full PE rate at moving-dim >= 256).
Layout is chosen so no on-device transposes are needed:
  - QK projection emits [f, tok] (q^T / k^T per head are direct slices)
  - V projection swaps matmul operands to emit v as [tok, f]
  - scores are computed transposed: sT[kt, qt]; softmax sums arrive via a
    ones-column appended to v in the AV matmul; masking is a 0/1 multiply
    after exp (exact, since exp>0 and rows are never fully masked).
  - normalization (1/l) is broadcast across partitions via a DRAM bounce
    and folded into the PSUM->SBUF copy of the attention output.
"""

import math

import numpy as np

import concourse.bass as bass
import concourse.mybir as mybir
import concourse.tile as tile
from concourse import bacc
from concourse.bass_utils import run_bass_kernel_spmd

N_CORES = 8
B, T, C = 16, 512, 1024
H = 16
DH = C // H  # 64
B_LOC = B // N_CORES  # 2
TOK = B_LOC * T  # 1024 tokens per core
P = 128
CT = C // P  # 8 contraction tiles
FQK = 2 * C  # q+k rows
DT = mybir.dt.float16
F32 = mybir.dt.float32


def _build_nc():
    nc = bacc.Bacc("TRN2", target_bir_lowering=False, debug=False,
                   num_devices=N_CORES)

    xT = nc.dram_tensor("xT", [C, TOK], DT, kind="ExternalInput").ap()
    wqkT = nc.dram_tensor("wqkT", [C, FQK], DT, kind="ExternalInput").ap()
    wvT = nc.dram_tensor("wvT", [C, C], DT, kind="ExternalInput").ap()
    woT = nc.dram_tensor("woT", [C, C], DT, kind="ExternalInput").ap()
    maskd = nc.dram_tensor("maskd", [T // P, P, P], DT,
                           kind="ExternalInput").ap()
    kpmb = nc.dram_tensor("kpmb", [B_LOC, T], F32, kind="ExternalInput").ap()
    bias = nc.dram_tensor("bias", [C], F32, kind="ExternalInput").ap()
    out = nc.dram_tensor("out", [TOK, C], F32, kind="ExternalOutput").ap()
    lall = nc.dram_tensor("lall", [B_LOC, H, T], F32).ap()
    linv_scr = nc.dram_tensor("linv_scr", [B_LOC, H, T], DT).ap()

    with tile.TileContext(nc) as tc:
        _emit(nc, tc, xT, wqkT, wvT, woT, maskd, kpmb, bias, out, lall,
              linv_scr)

    nc.compile()
    return nc


def _emit(nc, tc, xT, wqkT, wvT, woT, maskd, kpmb, bias, out, lall, linv_scr):
    from contextlib import ExitStack
    ctx = ExitStack()
    with ctx:
        singles = ctx.enter_context(tc.tile_pool(name="singles", bufs=1))
        wo_pool = ctx.enter_context(tc.tile_pool(name="wo", bufs=1))
        ps_a = ctx.enter_context(tc.tile_pool(name="ps_a", bufs=4, space="PSUM"))
        ps_s = ctx.enter_context(tc.tile_pool(name="ps_s", bufs=2, space="PSUM"))
        ps_o = ctx.enter_context(tc.tile_pool(name="ps_o", bufs=2, space="PSUM"))
        pt_pool = ctx.enter_context(tc.tile_pool(name="pt", bufs=2))
        linv_pool = ctx.enter_context(tc.tile_pool(name="linv", bufs=2))
        ao_stage_pool = ctx.enter_context(tc.tile_pool(name="aost", bufs=2))
        y_pool = ctx.enter_context(tc.tile_pool(name="y", bufs=3))

        NR = T // P  # 4 kt blocks

        # --- persistent SBUF tensors ---
        qk_sb = singles.tile([P, 16, TOK], DT)             # 32 KB/part
        v_sb = singles.tile([P, TOK // P, H, DH + 1], DT)  # 16.6 KB/part
        ao_b = [singles.tile([P, CT, T], DT, name=f"ao_b{b}")
                for b in range(B_LOC)]                     # 2x 8 KB/part

        bias_sb = singles.tile([P, C], F32)                # 4 KB/part
        maskd_sb = singles.tile([P, NR, P], DT)            # 1 KB/part
        kpmb_sb = singles.tile([P, B_LOC * NR], F32)

        with tc.tile_pool(name="xp", bufs=1) as x_pool, \
                tc.tile_pool(name="wq", bufs=3) as wq_pool, \
                tc.tile_pool(name="wv", bufs=1) as wv_pool:
            xk = [x_pool.tile([P, TOK], DT, tag=f"x_{k}", name=f"x_{k}")
                  for k in range(CT)]
            # first x chunk + first weight tile up front so the PE can start
            nc.sync.dma_start(out=xk[0][:], in_=xT[0:P, :])
            wq0 = wq_pool.tile([P, CT, P], DT, tag="wq", name="wq_0")
            nc.sync.dma_start(
                out=wq0[:],
                in_=wqkT[:, 0:P].rearrange("(k p) f -> p k f", p=P))
            for k in range(1, CT):
                nc.sync.dma_start(out=xk[k][:],
                                  in_=xT[k * P:(k + 1) * P, :])

            # --- phase 1: QK projection -> qk_sb[f, tok] ---
            for j in range(16):
                if j == 0:
                    wq = wq0
                else:
                    wq = wq_pool.tile([P, CT, P], DT, tag="wq",
                                      name=f"wq_{j}")
                    nc.sync.dma_start(
                        out=wq[:],
                        in_=wqkT[:, j * P:(j + 1) * P].rearrange(
                            "(k p) f -> p k f", p=P))
                ps = [ps_a.tile([P, 512], F32, tag="ps_a", name=f"ps_qk_{j}_{tt}")
                      for tt in range(2)]
                for k in range(CT):
                    for tt in range(2):
                        nc.tensor.matmul(ps[tt][:], wq[:, k, :],
                                         xk[k][:, tt * 512:(tt + 1) * 512],
                                         start=(k == 0), stop=(k == CT - 1))
                for tt in range(2):
                    nc.vector.tensor_copy(
                        out=qk_sb[:, j, tt * 512:(tt + 1) * 512],
                        in_=ps[tt][:])

            # --- phase 2: V projection -> v_sb[tok, h, d] (+ ones col) ---
            nc.vector.memset(v_sb[:, :, :, DH:DH + 1], 1.0)
            for n in range(2):
                wv = [wv_pool.tile([P, 512], DT, tag=f"wv_{k}",
                                   name=f"wv_{n}_{k}") for k in range(CT)]
                for k in range(CT):
                    nc.sync.dma_start(
                        out=wv[k][:],
                        in_=wvT[k * P:(k + 1) * P, n * 512:(n + 1) * 512])
                for m in range(TOK // P):
                    ps = ps_a.tile([P, 512], F32, tag="ps_a", name=f"ps_v_{n}_{m}")
                    for k in range(CT):
                        nc.tensor.matmul(
                            ps[:], xk[k][:, m * P:(m + 1) * P], wv[k][:],
                            start=(k == 0), stop=(k == CT - 1))
                    nc.vector.tensor_copy(
                        out=v_sb[:, m, 8 * n:8 * n + 8, 0:DH],
                        in_=ps[:].rearrange("p (h d) -> p h d", d=DH))

        # small constants + out-projection weights (prefetched here so they
        # don't delay the projection phase DMAs)
        nc.sync.dma_start(out=maskd_sb[:],
                          in_=maskd.rearrange("r p q -> p r q"))
        nc.sync.dma_start(out=kpmb_sb[:],
                          in_=kpmb.rearrange("b (r p) -> p (b r)", p=P))
        bias_bcast = bass.AP(tensor=bias.tensor, offset=bias.offset,
                             ap=[[0, P], *bias.ap])
        nc.gpsimd.dma_start(out=bias_sb[:], in_=bias_bcast)
        wo = [wo_pool.tile([P, 512], DT, tag=f"wo_{n}_{k}", name=f"wo_{n}_{k}")
              for n in range(2) for k in range(CT)]
        for n in range(2):
            for k in range(CT):
                nc.sync.dma_start(
                    out=wo[n * CT + k][:],
                    in_=woT[k * P:(k + 1) * P, n * 512:(n + 1) * 512])

        def yproj_chunk(b, i):
            n, m = i // (T // P), i % (T // P)
            ps = ps_a.tile([P, 512], F32, tag="ps_a", name=f"ps_y_{b}_{n}_{m}")
            for k in range(CT):
                nc.tensor.matmul(ps[:], ao_b[b][:, k, m * P:(m + 1) * P],
                                 wo[n * CT + k][:],
                                 start=(k == 0), stop=(k == CT - 1))
            y = y_pool.tile([P, 512], F32, tag="y")
            nc.vector.tensor_add(out=y[:], in0=ps[:],
                                 in1=bias_sb[:, n * 512:(n + 1) * 512])
            nc.sync.dma_start(
                out=out[b * T + m * P: b * T + (m + 1) * P,
                        n * 512:(n + 1) * 512],
                in_=y[:])

        # --- phase 3+4 interleaved per batch: b0 attention, then b1
        # attention with b0's out-projection chunks woven between heads ---
        for b in range(B_LOC):
            for h in range(H):
                jq, jk, dlo = h // 2, 8 + h // 2, DH * (h % 2)
                pT = pt_pool.tile([P, NR, 512], DT, tag="pT")
                sT = [ps_s.tile([P, 512], F32, tag="sT", name=f"sT_{b}_{h}_{r}")
                      for r in range(NR)]
                for r in range(NR):
                    kT = qk_sb[dlo:dlo + DH, jk,
                               b * T + r * P: b * T + (r + 1) * P]
                    qTr = qk_sb[dlo:dlo + DH, jq,
                                b * T + r * P:(b + 1) * T]
                    nc.tensor.matmul(sT[r][:, r * P:], kT, qTr,
                                     start=True, stop=True)
                    # exp over the un-masked tail; key-padding enters as an
                    # additive bias (0 or -1e30) per kt partition
                    nc.scalar.activation(
                        out=pT[:, r, r * P:], in_=sT[r][:, r * P:],
                        func=mybir.ActivationFunctionType.Exp,
                        bias=kpmb_sb[:, b * NR + r: b * NR + r + 1])
                    # causal mask inside the diagonal block only
                    nc.vector.tensor_mul(
                        out=pT[:, r, r * P:(r + 1) * P],
                        in0=pT[:, r, r * P:(r + 1) * P],
                        in1=maskd_sb[:, r, :])
                po = ps_o.tile([P, 512], F32, tag="po")
                for r in range(NR):
                    # masked columns of pT are never read: slice rhs/out
                    nc.tensor.matmul(po[0:DH + 1, r * P:],
                                     v_sb[:, b * NR + r, h, :],
                                     pT[:, r, r * P:],
                                     start=(r == 0), stop=(r == NR - 1))
                # stash row sums; normalization is batched per b
                lrow = linv_pool.tile([P, 512], F32, tag="lrow")
                nc.vector.tensor_copy(out=lrow[DH:DH + 1, :],
                                      in_=po[DH:DH + 1, :])
                nc.sync.dma_start(out=lall[b, h, :], in_=lrow[DH:DH + 1, :])
                if dlo == 0:
                    nc.vector.tensor_copy(
                        out=ao_b[b][0:DH, jq, :], in_=po[0:DH, :])
                else:
                    ao_st = ao_stage_pool.tile([DH, 512], DT, tag="ao_st")
                    nc.vector.tensor_copy(out=ao_st[:], in_=po[0:DH, :])
                    nc.sync.dma_start(out=ao_b[b][dlo:dlo + DH, jq, :],
                                      in_=ao_st[:])
                if b == 1 and h % 2 == 1:
                    yproj_chunk(0, h // 2)
                if h % (H // 2) == H // 2 - 1:
                    # 1/l for the finished half of the heads, then in-place
                    # normalize the corresponding ao c-tiles
                    half = h // (H // 2)
                    hs = slice(half * (H // 2), (half + 1) * (H // 2))
                    lpart = linv_pool.tile([H // 2, T], F32, tag="lpart",
                                           name=f"lpart_{b}_{half}")
                    nc.sync.dma_start(out=lpart[:], in_=lall[b, hs])
                    nc.vector.reciprocal(out=lpart[:], in_=lpart[:])
                    lpartd = linv_pool.tile([H // 2, T], DT, tag="lpartd",
                                            name=f"lpartd_{b}_{half}")
                    nc.vector.tensor_copy(out=lpartd[:], in_=lpart[:])
                    nc.sync.dma_start(out=linv_scr[b, hs], in_=lpartd[:])
                    for k in range(half * (CT // 2), (half + 1) * (CT // 2)):
                        lf = linv_pool.tile([P, T], DT, tag="lf")
                        for hf in range(2):
                            hh = 2 * k + hf
                            src_ap = bass.AP(
                                tensor=linv_scr.tensor,
                                offset=linv_scr.offset + (b * H + hh) * T,
                                ap=[[0, DH], [1, T]])
                            nc.sync.dma_start(
                                out=lf[hf * DH:(hf + 1) * DH, :], in_=src_ap)
                        nc.vector.tensor_mul(out=ao_b[b][:, k, :],
                                             in0=ao_b[b][:, k, :], in1=lf[:])

        for i in range(2 * (T // P)):
            yproj_chunk(1, i)





_NC_CACHE = None


def _get_nc():
    global _NC_CACHE
    if _NC_CACHE is None:
        _NC_CACHE = _build_nc()
    return _NC_CACHE


def _prep_core_inputs(x, mask, key_padding_mask, w_qkv, w_out, b_out):
    """Host-side sharding + layout prep. Returns list of per-core in_maps."""
    x = np.asarray(x, dtype=np.float32)
    mask = np.asarray(mask)
    kpm = np.asarray(key_padding_mask)
    w_qkv = np.asarray(w_qkv, dtype=np.float32)
    w_out = np.asarray(w_out, dtype=np.float32)
    b_out = np.asarray(b_out, dtype=np.float32)

    scale = 1.0 / math.sqrt(DH)
    wqkT = w_qkv[:FQK].T.copy()  # [C, 2C]
    wqkT[:, :C] *= scale  # fold 1/sqrt(dh) into the Q weights
    wqkT = wqkT.astype(np.float16)
    wvT = np.ascontiguousarray(w_qkv[FQK:].T.astype(np.float16))  # [C, C]
    woT = np.ascontiguousarray(w_out.T.astype(np.float16))        # [C, C]

    # The kernel exploits the causal structure: it only applies mask values
    # inside the diagonal 128x128 blocks and zero-fills fully-masked blocks.
    # Verify the input mask really is lower-triangular.
    NRl = T // P
    exp_tril = np.tril(np.ones((T, T), dtype=mask.dtype))
    assert np.array_equal(mask, exp_tril), "kernel assumes causal tril mask"
    maskTf = mask.T.astype(np.float16)  # [kt, qt]
    maskd = np.stack([maskTf[r * P:(r + 1) * P, r * P:(r + 1) * P]
                      for r in range(NRl)])  # [NR, P, P]

    in_maps = []
    for i in range(N_CORES):
        xs = x[i * B_LOC:(i + 1) * B_LOC]      # [B_LOC, T, C]
        xT = np.ascontiguousarray(xs.reshape(TOK, C).T.astype(np.float16))
        kb = np.where(kpm[i * B_LOC:(i + 1) * B_LOC], -1e30,
                      0.0).astype(np.float32)  # [B_LOC, T]
        in_maps.append({
            "xT": xT,
            "wqkT": wqkT,
            "wvT": wvT,
            "woT": woT,
            "maskd": np.ascontiguousarray(maskd),
            "kpmb": kb,
            "bias": b_out,
        })
    return in_maps


def kernel(x, mask, key_padding_mask, w_qkv, w_out, b_out, _trace=False,
           _tmpdir=None):
    nc = _get_nc()
    in_maps = _prep_core_inputs(x, mask, key_padding_mask, w_qkv, w_out, b_out)
    res = run_bass_kernel_spmd(nc, in_maps, list(range(N_CORES)),
                               trace=_trace, tmpdir=_tmpdir)
    outs = [res.results[i]["out"].reshape(B_LOC, T, C) for i in range(N_CORES)]
    full = np.concatenate(outs, axis=0).astype(np.float32)
    kernel._last_exec_time_ns = res.exec_time_ns
    return full
